# revision 1
# baseline (speedup 1.0000x reference)
"""FAVOR+ (Performer linear attention) Trainium2 Bass kernel.

Full inputs: keys/values/queries (16, 128, 8192) fp32, features (128, 128) fp32.
Data-parallel over batch: 2 batches per core x 8 cores.

Per batch b (D = Dv = M = 128, L = 8192):
  phi_x = relu(x^T W) / sqrt(M)                 (L, M)
  KV    = [phi_k^T V^T | phi_k^T 1]             (M, Dv+1)   pass 1
  phiT_q = relu(W^T q)                          (M, L)      pass 2
  den    = ksum^T phiT_q                        (1, L)
  outT   = (KV[:, :Dv])^T phiT_q * (1/den bcast)  (Dv, L) == output layout

Pass 1 runs per 128-wide l-tile: phi_k via matmul(lhsT=K_tile, rhs=W),
V_tile transposed on the PE (identity), then kv accumulated into one PSUM
bank across all 64 l-tiles.  Pass 2 is all N=512 streaming matmuls with
stationary weights reused across 16 chunks (W, ksum, ones-row, KV).
1/den is broadcast across partitions with a contraction-1 outer-product
matmul, reciprocal on the DVE, final normalize fused into the PSUM->SBUF
copy before the output DMA.
"""

import math

import numpy as np

import concourse.bacc as bacc
import concourse.mybir as mybir
import concourse.tile as tile
from concourse import bass_utils

F32 = mybir.dt.float32
RELU = mybir.ActivationFunctionType.Relu

B_FULL = 16
N_CORES = 8
B_PER_CORE = B_FULL // N_CORES  # 2
D = 128     # key/query feature dim
DV = 128    # value dim
M = 128     # random features
L = 8192    # sequence length
PHI_SCALE = 1.0 / math.sqrt(M)

CHUNK = 512
N_CHUNKS = L // CHUNK   # 16
TILES_PER_CHUNK = CHUNK // 128  # 4
N_TILES = L // 128      # 64


def _build_kernel(nc):
    keys_d = nc.dram_tensor("keys", (B_PER_CORE, D, L), F32, kind="ExternalInput").ap()
    vals_d = nc.dram_tensor("values", (B_PER_CORE, DV, L), F32, kind="ExternalInput").ap()
    qrys_d = nc.dram_tensor("queries", (B_PER_CORE, D, L), F32, kind="ExternalInput").ap()
    feat_d = nc.dram_tensor("features", (D, M), F32, kind="ExternalInput").ap()
    ident_d = nc.dram_tensor("ident", (128, 128), F32, kind="ExternalInput").ap()
    out_d = nc.dram_tensor("out", (B_PER_CORE, DV, L), F32, kind="ExternalOutput").ap()

    with tile.TileContext(nc) as tc:
        with (
            tc.tile_pool(name="const", bufs=1) as const_pool,
            tc.tile_pool(name="inp", bufs=3) as in_pool,
            tc.tile_pool(name="small", bufs=3) as small_pool,
            tc.tile_pool(name="big", bufs=1) as big_pool,
            tc.tile_pool(name="osb", bufs=3) as out_pool,
            tc.tile_pool(name="ps_phi", bufs=2, space="PSUM") as ps_phi,
            tc.tile_pool(name="ps_vt", bufs=2, space="PSUM") as ps_vt,
            tc.tile_pool(name="ps_kv", bufs=2, space="PSUM") as ps_kv,
            tc.tile_pool(name="ps_t2", bufs=2, space="PSUM") as ps_t2,
        ):
            w_sb = const_pool.tile([D, M], F32, tag="w")
            nc.sync.dma_start(w_sb[:], feat_d)
            id_sb = const_pool.tile([128, 128], F32, tag="id")
            nc.sync.dma_start(id_sb[:], ident_d)
            ones_row = const_pool.tile([1, 128], F32, tag="ones")
            nc.vector.memset(ones_row[:], 1.0)

            for b in range(B_PER_CORE):
                # ---------------- pass 1: KV = [phi_k^T V^T | ksum] ----------------
                kv_ps = ps_kv.tile([M, DV + 1], F32, tag="kv")
                for c in range(N_CHUNKS):
                    k_sb = in_pool.tile([D, CHUNK], F32, tag="k_in")
                    nc.sync.dma_start(k_sb[:], keys_d[b, :, c * CHUNK:(c + 1) * CHUNK])
                    v_sb = in_pool.tile([DV, CHUNK], F32, tag="v_in")
                    nc.sync.dma_start(v_sb[:], vals_d[b, :, c * CHUNK:(c + 1) * CHUNK])
                    for t in range(TILES_PER_CHUNK):
                        i = c * TILES_PER_CHUNK + t
                        ksl = k_sb[:, t * 128:(t + 1) * 128]
                        vsl = v_sb[:, t * 128:(t + 1) * 128]
                        phi_ps = ps_phi.tile([128, M], F32, tag="phi")
                        nc.tensor.matmul(phi_ps[:], ksl, w_sb[:], start=True, stop=True)
                        phi_sb = small_pool.tile([128, M], F32, tag="phi_sb")
                        nc.scalar.activation(phi_sb[:], phi_ps[:], RELU, scale=PHI_SCALE)
                        vt_ps = ps_vt.tile([128, DV], F32, tag="vt")
                        nc.tensor.transpose(vt_ps[:], vsl, id_sb[:])
                        vt_sb = small_pool.tile([128, DV + 1], F32, tag="vt_sb")
                        nc.vector.tensor_copy(vt_sb[:, 0:DV], vt_ps[:])
                        nc.vector.memset(vt_sb[:, DV:DV + 1], 1.0)
                        nc.tensor.matmul(
                            kv_ps[:], phi_sb[:], vt_sb[:],
                            start=(i == 0), stop=(i == N_TILES - 1),
                        )
                kv_sb = small_pool.tile([M, DV + 1], F32, tag="kv_sb")
                nc.scalar.copy(kv_sb[:], kv_ps[:])

                # ---------------- pass 2a: phiT_q (M, L) ----------------
                phiq = big_pool.tile([M, L], F32, tag="phiq")
                for c in range(N_CHUNKS):
                    q_sb = in_pool.tile([D, CHUNK], F32, tag="q_in")
                    nc.sync.dma_start(q_sb[:], qrys_d[b, :, c * CHUNK:(c + 1) * CHUNK])
                    pq_ps = ps_t2.tile([M, CHUNK], F32, tag="t2")
                    nc.tensor.matmul(pq_ps[:], w_sb[:], q_sb[:], start=True, stop=True)
                    nc.scalar.activation(
                        phiq[:, c * CHUNK:(c + 1) * CHUNK], pq_ps[:], RELU,
                        scale=PHI_SCALE,
                    )

                # ---------------- pass 2b: den (1, L) ----------------
                den_sb = big_pool.tile([1, L], F32, tag="den")
                for c in range(N_CHUNKS):
                    den_ps = ps_t2.tile([1, CHUNK], F32, tag="t2")
                    nc.tensor.matmul(
                        den_ps[:], kv_sb[:, DV:DV + 1],
                        phiq[:, c * CHUNK:(c + 1) * CHUNK], start=True, stop=True,
                    )
                    nc.scalar.copy(den_sb[:, c * CHUNK:(c + 1) * CHUNK], den_ps[:])

                # ---------------- pass 2c: recip = 1 / bcast(den) ----------------
                recip = big_pool.tile([128, L], F32, tag="recip")
                for c in range(N_CHUNKS):
                    bc_ps = ps_t2.tile([128, CHUNK], F32, tag="t2")
                    nc.tensor.matmul(
                        bc_ps[:], ones_row[:],
                        den_sb[:, c * CHUNK:(c + 1) * CHUNK], start=True, stop=True,
                    )
                    nc.vector.reciprocal(recip[:, c * CHUNK:(c + 1) * CHUNK], bc_ps[:])

                # ---------------- pass 2d: outT, normalize, store ----------------
                for c in range(N_CHUNKS):
                    o_ps = ps_t2.tile([DV, CHUNK], F32, tag="t2")
                    nc.tensor.matmul(
                        o_ps[:], kv_sb[:, 0:DV],
                        phiq[:, c * CHUNK:(c + 1) * CHUNK], start=True, stop=True,
                    )
                    o_sb = out_pool.tile([DV, CHUNK], F32, tag="o_sb")
                    nc.vector.tensor_mul(
                        o_sb[:], o_ps[:], recip[:, c * CHUNK:(c + 1) * CHUNK]
                    )
                    nc.sync.dma_start(out_d[b, :, c * CHUNK:(c + 1) * CHUNK], o_sb[:])

    nc.compile()
    return nc


_NC_CACHE = None


def _get_nc():
    global _NC_CACHE
    if _NC_CACHE is None:
        nc = bacc.Bacc("TRN2", target_bir_lowering=False, debug=False)
        _NC_CACHE = _build_kernel(nc)
    return _NC_CACHE


def _make_in_maps(keys, values, queries, features):
    ident = np.eye(128, dtype=np.float32)
    feats = np.ascontiguousarray(features, dtype=np.float32)
    in_maps = []
    for i in range(N_CORES):
        sl = slice(i * B_PER_CORE, (i + 1) * B_PER_CORE)
        in_maps.append({
            "keys": np.ascontiguousarray(keys[sl], dtype=np.float32),
            "values": np.ascontiguousarray(values[sl], dtype=np.float32),
            "queries": np.ascontiguousarray(queries[sl], dtype=np.float32),
            "features": feats,
            "ident": ident,
        })
    return in_maps


def run(keys, values, queries, features, trace=False):
    nc = _get_nc()
    in_maps = _make_in_maps(keys, values, queries, features)
    res = bass_utils.run_bass_kernel_spmd(
        nc, in_maps, core_ids=list(range(N_CORES)), trace=trace,
    )
    out = np.concatenate([r["out"] for r in res.results], axis=0)
    return out.astype(np.float32, copy=False), res


def kernel(keys, values, queries, features):
    out, _ = run(
        np.asarray(keys), np.asarray(values), np.asarray(queries),
        np.asarray(features),
    )
    return out
